# revision 25
# baseline (speedup 1.0000x reference)
"""Multi-head attention (B=4, S=2048, D=1024, H=16, causal, all-valid padding)
for 8 Trainium2 NeuronCores.

Sharding: hybrid data-parallel x tensor-parallel. Core c handles batch
b = c // 2 and head-group g = c % 2 (8 heads, 512 channels each). Each core
computes its head-group's Q/K/V projections, causal attention, and the
partial output projection through its slice of Wo. The host sums the two
head-group partials per batch (the row-parallel all-reduce) and stacks
batches.

On-chip layout (per core):
  - x fed pre-transposed and pre-tiled [128, sc, d, s] so every DMA moves
    8KB-contiguous per-partition lines (full HBM bandwidth).
  - Q^T, K^T kept as [128ch, S] tiles (two 64-ch heads stacked per pair) so
    scores are computed transposed: S^T[k,q] = K_tile @ Q^T; the two heads
    run as concurrent 64-row PE tiles.
  - P^T = exp(S^T/8) via ACT straight out of PSUM, causal-masked by a
    precomputed bf16 mask multiply on DVE. Diagonal tiles only compute the
    valid column range; fully masked tiles are skipped.
  - ctx^T accumulates in PSUM via col-group-concurrent V-matmuls; softmax
    denominators accumulate pre-broadcast in a parallel bank via an
    all-ones stationary operand.
  - The attention k-tile loop is ACT(exp)-throughput-bound at ~1000ns per
    k-tile while the PE only needs ~800ns. QKV/Wo matmuls are queued as
    single-matmul filler items in two queues: DVE-light QKV items drip one
    per off-diagonal k-tile (placed between scores(i+1) and ctx(i)); Wo
    items (whose f32 PSUM copy would queue on the DVE ahead of the
    pair-end normalization and stretch attention) pop only at pair starts
    (~3072 cols, covering the exp-backlog/normalization bubble) and at
    chunk-boundary flushes, where their PSUM->SBUF copy runs on the then-
    idle ACT engine instead of the DVE.
  - Normalization is ordered for early PSUM-bank release: copy raw ctx out
    (frees the ctx bank), reciprocal (frees den), multiply in SBUF off the
    critical path.
"""

import numpy as np
import ml_dtypes
from collections import deque

B, S, D, H = 4, 2048, 1024, 16
DK = D // H            # 64
CH = D // 2            # 512 local channels per core (8 heads)
NPAIR = 4              # pairs of heads per core (2 heads x 64ch = 128ch tile)
SCHUNK = 512           # s-chunk (q-chunk) width
DRIP = True            # drip one filler matmul per off-diagonal k-tile
PAIR_FILL = 3072       # filler columns popped at each pair start
KTILE = 128            # k-tile width
NDT = D // 128         # 8 d-tiles (contraction for projections)
NSC = S // SCHUNK      # 4 s-chunks

_BF16 = ml_dtypes.bfloat16


def _build_nc(s_len):
    import concourse.mybir as mybir
    import concourse.tile as tile
    from concourse import bacc

    f32 = mybir.dt.float32
    bf16 = mybir.dt.bfloat16
    Exp = mybir.ActivationFunctionType.Exp

    nsc = s_len // SCHUNK          # s-chunks / q-chunks
    nkt_total = s_len // KTILE     # k-tiles

    nc = bacc.Bacc("TRN2", target_bir_lowering=False, debug=False)

    # host pre-tiles everything so each DMA line is >=2KB contiguous
    xq_d = nc.dram_tensor("xqT", [128, nsc, NDT, SCHUNK], bf16,
                          kind="ExternalInput")
    xk_d = nc.dram_tensor("xkT", [128, nsc, NDT, SCHUNK], bf16,
                          kind="ExternalInput")
    xv_d = nc.dram_tensor("xvT", [128, nsc, NDT, SCHUNK], bf16,
                          kind="ExternalInput")
    wq_d = nc.dram_tensor("wqT", [128, NPAIR, NDT, 128], bf16,
                          kind="ExternalInput")
    wk_d = nc.dram_tensor("wkT", [128, NDT, CH], bf16, kind="ExternalInput")
    wv_d = nc.dram_tensor("wvT", [128, NDT, CH], bf16, kind="ExternalInput")
    wo_d = nc.dram_tensor("woT", [128, NPAIR, D], bf16, kind="ExternalInput")
    mask_d = nc.dram_tensor("masks", [4, 128, SCHUNK], bf16,
                            kind="ExternalInput")
    y_d = nc.dram_tensor("y", [s_len, D], bf16, kind="ExternalOutput")

    x_d = {"q": xq_d, "k": xk_d, "v": xv_d}

    with tile.TileContext(nc) as tc:
        from contextlib import ExitStack

        with ExitStack() as ctx:
            const_pool = ctx.enter_context(tc.tile_pool(name="const", bufs=1))
            w_pool = ctx.enter_context(tc.tile_pool(name="weights", bufs=1))
            qt_pool = ctx.enter_context(tc.tile_pool(name="qt", bufs=NPAIR * nsc))
            kt_pool = ctx.enter_context(tc.tile_pool(name="kt", bufs=NPAIR * nsc))
            v_pool = ctx.enter_context(tc.tile_pool(name="v", bufs=nkt_total))
            ctx_pool = ctx.enter_context(tc.tile_pool(name="ctx", bufs=NPAIR * nsc))
            x_pool = ctx.enter_context(tc.tile_pool(name="x", bufs=12))
            pt_pool = ctx.enter_context(tc.tile_pool(name="pt", bufs=4))
            ev_pool = ctx.enter_context(tc.tile_pool(name="ev", bufs=4))
            y_pool = ctx.enter_context(tc.tile_pool(name="yout", bufs=3))
            # PSUM: 8 banks exactly: qkv 2 + st 2x2 + ctx 1 + den 1
            qkv_ps = ctx.enter_context(
                tc.tile_pool(name="qkv_ps", bufs=2, space="PSUM"))
            st_ps = ctx.enter_context(
                tc.tile_pool(name="st_ps", bufs=2, space="PSUM"))
            ctx_ps_pool = ctx.enter_context(
                tc.tile_pool(name="ctx_ps", bufs=1, space="PSUM"))
            den_ps_pool = ctx.enter_context(
                tc.tile_pool(name="den_ps", bufs=1, space="PSUM"))

            # x tiles split in d-halves so the first projection matmuls can
            # start after half a chunk has landed
            x_tiles = {}

            def one_x_dma(key, sc):
                ta = x_pool.tile([128, NDT // 2, SCHUNK], bf16, tag="x",
                                 name=f"x{key}_{sc}a")
                tb = x_pool.tile([128, NDT // 2, SCHUNK], bf16, tag="x",
                                 name=f"x{key}_{sc}b")
                nc.sync.dma_start(ta[:, :, :], x_d[key][:, sc, 0:NDT // 2, :])
                nc.sync.dma_start(tb[:, :, :], x_d[key][:, sc, NDT // 2:, :])
                x_tiles[(key, sc)] = (ta, tb)

            def issue_x_dma(sc):
                for key in ("q", "k", "v"):
                    one_x_dma(key, sc)

            # ones first (memset, no DMA): feeds den matmuls + PE warmup
            ones_sb = const_pool.tile([128, 64], bf16)
            nc.vector.memset(ones_sb[:, :], 1.0)

            # DMA issue order = first-use order for the startup pipeline
            wq_sb = []
            for m in range(NPAIR):
                t = w_pool.tile([128, NDT, 128], bf16, name=f"wq_{m}")
                wq_sb.append(t)
            nc.sync.dma_start(wq_sb[0][:, :, :], wq_d[:, 0, :, :])
            ta = x_pool.tile([128, NDT // 2, SCHUNK], bf16, tag="x",
                             name="xq_0a")
            tb = x_pool.tile([128, NDT // 2, SCHUNK], bf16, tag="x",
                             name="xq_0b")
            nc.sync.dma_start(ta[:, 0:2, :], xq_d[:, 0, 0:2, :])
            nc.sync.dma_start(ta[:, 2:4, :], xq_d[:, 0, 2:4, :])
            nc.sync.dma_start(tb[:, :, :], xq_d[:, 0, NDT // 2:, :])
            x_tiles[("q", 0)] = (ta, tb)
            # warm the PE HAM on the memset constant while DMAs land so the
            # first real burst runs at 2.4GHz (discarded output)
            warm_ps = qkv_ps.tile([128, 64], f32, tag="qkv", name="warm")
            for i in range(22):
                nc.tensor.matmul(warm_ps[0:64, :], lhsT=ones_sb[:, :],
                                 rhs=ones_sb[:, :],
                                 start=(i == 0), stop=(i == 21))
            # wq m=1..3 early: the qkv(0) flush pops q(m0..3) first
            for m in range(1, NPAIR):
                nc.sync.dma_start(wq_sb[m][:, :, :], wq_d[:, m, :, :])
            wk_sb = w_pool.tile([128, NDT, CH], bf16)
            nc.sync.dma_start(wk_sb[:, :, :], wk_d[:, :, :])
            one_x_dma("k", 0)
            wv_sb = w_pool.tile([128, NDT, CH], bf16)
            nc.sync.dma_start(wv_sb[:, :, :], wv_d[:, :, :])
            one_x_dma("v", 0)
            mask_sb = const_pool.tile([128, 4, SCHUNK], bf16)
            nc.sync.dma_start(
                mask_sb[:, :, :], mask_d[:, :, :].rearrange("r p m -> p r m"))
            # chunk 1 x prefetched up front so fillers are ready during qc=0
            if nsc > 1:
                issue_x_dma(1)
            wo_sb = w_pool.tile([128, NPAIR, D], bf16)
            nc.sync.dma_start(wo_sb[:, :, :], wo_d[:, :, :])

            qt_tiles = {}
            kt_tiles = {}
            v_tiles = {}
            ctx_tiles = {}

            # deferred-work queue of fine-grained PE items:
            # (tag, cols, closure). Popped between scores(i+1) and ctx(i)
            # inside the attention k-tile loop so the PE never idles while
            # the ACT engine paces the softmax pipeline.
            # two queues: qkv items are DVE-light (one bf16 copy per 8
            # items) and safe to drip mid-attention; wo items carry an f32
            # PSUM copy + y DMA, which would queue on the DVE ahead of the
            # pair-end normalization and stretch attention, so they only
            # pop at pair starts and flushes.
            fq_qkv = deque()
            fq_wo = deque()
            debt = [0]
            flushing = [False]

            def emit_drip(budget_cols):
                debt[0] += budget_cols
                while fq_qkv and debt[0] >= fq_qkv[0][1]:
                    tag, cols, fn = fq_qkv.popleft()
                    debt[0] -= cols
                    fn()

            def emit_pair_fill(budget_cols):
                budget = budget_cols
                while fq_wo and budget >= fq_wo[0][1]:
                    tag, cols, fn = fq_wo.popleft()
                    budget -= cols
                    fn()
                debt[0] += budget
                while fq_qkv and debt[0] >= fq_qkv[0][1]:
                    tag, cols, fn = fq_qkv.popleft()
                    debt[0] -= cols
                    fn()

            def flush_tag(tag):
                while fq_qkv and fq_qkv[0][0] == tag:
                    fq_qkv.popleft()[2]()
                debt[0] = 0

            def flush_all():
                flushing[0] = True
                while fq_wo:
                    fq_wo.popleft()[2]()
                while fq_qkv:
                    fq_qkv.popleft()[2]()
                debt[0] = 0
                flushing[0] = False

            def qkv_items(kind, m, sc):
                """8 single-matmul filler items; the last also copies
                PSUM -> SBUF. One item (~213ns) fits the per-ktile PE slack
                under the ACT-paced attention pipeline."""
                state = {}

                def item(dd):
                    def emit():
                        if dd == 0:
                            width = CH if kind == "v" else SCHUNK
                            state["ps"] = qkv_ps.tile(
                                [128, width], f32, tag="qkv",
                                name=f"qkvps_{kind}_{m}_{sc}")
                        ps = state["ps"]
                        xa, xb = x_tiles[(kind, sc)]
                        xt = xa if dd < NDT // 2 else xb
                        di = dd if dd < NDT // 2 else dd - NDT // 2
                        if kind == "q":
                            nc.tensor.matmul(
                                ps[:, :], lhsT=wq_sb[m][:, dd, :],
                                rhs=xt[:, di, :],
                                start=(dd == 0), stop=(dd == NDT - 1))
                        elif kind == "k":
                            nc.tensor.matmul(
                                ps[:, :],
                                lhsT=wk_sb[:, dd, m * 128:(m + 1) * 128],
                                rhs=xt[:, di, :],
                                start=(dd == 0), stop=(dd == NDT - 1))
                        else:
                            nc.tensor.matmul(
                                ps[:, :],
                                lhsT=xt[:, di, m * 128:(m + 1) * 128],
                                rhs=wv_sb[:, dd, :],
                                start=(dd == 0), stop=(dd == NDT - 1))
                        if dd == NDT - 1:
                            if kind == "q":
                                t = qt_pool.tile([128, SCHUNK], bf16,
                                                 tag="qt", name=f"qt_{m}_{sc}")
                                nc.vector.tensor_copy(t[:, :], ps[:, :])
                                qt_tiles[(m, sc)] = t
                            elif kind == "k":
                                t = kt_pool.tile([128, SCHUNK], bf16,
                                                 tag="kt", name=f"kt_{m}_{sc}")
                                nc.vector.tensor_copy(t[:, :], ps[:, :])
                                kt_tiles[(m, sc)] = t
                            else:
                                kt_idx = sc * (SCHUNK // 128) + m
                                t = v_pool.tile([128, CH], bf16, tag="v",
                                                name=f"v_{kt_idx}")
                                nc.vector.tensor_copy(t[:, :], ps[:, :])
                                v_tiles[kt_idx] = t
                    return emit

                return [item(dd) for dd in range(NDT)]

            def push_qkv(sc):
                for kind in ("q", "k", "v"):
                    for m in range(NPAIR):
                        for it in qkv_items(kind, m, sc):
                            fq_qkv.append(((("qkv", sc)), 512, it))

            def wo_items(qt, oc, qc):
                state = {}

                def item(cj):
                    def emit():
                        if cj == 0:
                            state["ps"] = qkv_ps.tile(
                                [128, 512], f32, tag="qkv",
                                name=f"wops_{qt}_{oc}")
                        ps = state["ps"]
                        nc.tensor.matmul(
                            ps[:, :],
                            lhsT=ctx_tiles[(cj, qc)][:, (qt % 4) * 128:
                                                     (qt % 4 + 1) * 128],
                            rhs=wo_sb[:, cj, oc * 512:(oc + 1) * 512],
                            start=(cj == 0), stop=(cj == NPAIR - 1))
                        if cj == NPAIR - 1:
                            yt = y_pool.tile([128, 512], bf16, tag="yout")
                            if flushing[0]:
                                # ACT is idle during GEMM bursts/tail; DVE
                                # may still be draining normalization work
                                nc.scalar.copy(yt[:, :], ps[:, :])
                            else:
                                nc.vector.tensor_copy(yt[:, :], ps[:, :])
                            nc.sync.dma_start(
                                y_d[qt * 128:(qt + 1) * 128,
                                    oc * 512:(oc + 1) * 512],
                                yt[:, :])
                    return emit

                return [item(cj) for cj in range(NPAIR)]

            def push_wo(qc):
                for qt in range(qc * 4, (qc + 1) * 4):
                    for oc in range(D // 512):
                        for it in wo_items(qt, oc, qc):
                            fq_wo.append((("wo", qc), 512, it))

            push_qkv(0)
            for sc in range(nsc):
                # flush first: guarantees all x(sc) readers are emitted
                # before x(sc+2)'s DMA reuses their buffers (WAR ordering)
                flush_tag(("qkv", sc))
                if sc + 2 < nsc:
                    issue_x_dma(sc + 2)
                if sc + 1 < nsc:
                    push_qkv(sc + 1)

                # ---- attention for q-chunk qc = sc ----
                qc = sc
                nkt = (qc + 1) * (SCHUNK // KTILE)  # causal: k-tiles 0..nkt-1
                for pair in range(NPAIR):
                    ctx_p = ctx_ps_pool.tile([128, SCHUNK], f32, tag="ctxps")
                    den_p = den_ps_pool.tile([128, SCHUNK], f32, tag="denps")

                    def tile_off(kt):
                        # diagonal tile r: columns [0, 128r) are fully masked
                        r = kt - qc * (SCHUNK // KTILE)
                        return 128 * r if r > 0 else 0

                    def emit_scores(kt):
                        off = tile_off(kt)
                        st = st_ps.tile([128, 2 * SCHUNK], f32, tag="st",
                                        name="st")
                        ktile = kt_tiles[(pair, kt // 4)]
                        qtile = qt_tiles[(pair, qc)]
                        for h in range(2):
                            nc.tensor.matmul(
                                st[:, h * SCHUNK + off:(h + 1) * SCHUNK],
                                lhsT=ktile[h * 64:(h + 1) * 64,
                                           (kt % 4) * KTILE:(kt % 4 + 1) * KTILE],
                                rhs=qtile[h * 64:(h + 1) * 64, off:],
                                start=True, stop=True)
                        pt = pt_pool.tile([128, 2 * SCHUNK], bf16, tag="pt")
                        # per-head exp instructions (NOT fused across heads:
                        # a fused strided instruction saves ~260ns of ACT
                        # time per diagonal tile but makes ctx_h0 wait for
                        # both heads' exp, which costs more in chain latency
                        # at every pair end -- measured +12us)
                        if off == 0:
                            nc.scalar.activation(pt[:, :], st[:, :], Exp,
                                                 scale=0.125)
                        else:
                            for h in range(2):
                                nc.scalar.activation(
                                    pt[:, h * SCHUNK + off:(h + 1) * SCHUNK],
                                    st[:, h * SCHUNK + off:(h + 1) * SCHUNK],
                                    Exp, scale=0.125)
                        r = kt - qc * (SCHUNK // KTILE)
                        if r >= 0:  # diagonal tile: apply causal mask
                            for h in range(2):
                                nc.vector.tensor_mul(
                                    pt[:, h * SCHUNK + off:(h + 1) * SCHUNK],
                                    pt[:, h * SCHUNK + off:(h + 1) * SCHUNK],
                                    mask_sb[:, r, off:])
                        return pt

                    pt_cur = emit_scores(0)
                    # pair-start filler burst AFTER the first scores: the
                    # ACT engine starts the pair's exp chain immediately
                    # (no ~1.4us bubble) while the burst runs under exp(0)
                    emit_pair_fill(PAIR_FILL)
                    for kt in range(nkt):
                        pt_next = emit_scores(kt + 1) if kt + 1 < nkt else None
                        # drip ONE 512-col filler matmul per off-diagonal
                        # k-tile, placed between scores(i+1) and ctx(i): it
                        # fits the ~200ns PE slack under the ~1000ns
                        # ACT-paced k-tile period without delaying the
                        # scores->exp chain (the pacer), so it absorbs GEMM
                        # work at near-zero marginal cost.
                        if DRIP and kt // 4 < qc:
                            emit_drip(512)
                        off = tile_off(kt)
                        vt = v_tiles[kt]
                        for h in range(2):
                            hl = pair * 2 + h
                            nc.tensor.matmul(
                                ctx_p[h * 64:(h + 1) * 64, off:],
                                lhsT=vt[:, hl * 64:(hl + 1) * 64],
                                rhs=pt_cur[:, h * SCHUNK + off:(h + 1) * SCHUNK],
                                start=(kt == 0), stop=(kt == nkt - 1),
                                tile_position=(0, h * 64),
                                skip_group_check=True)
                            nc.tensor.matmul(
                                den_p[h * 64:(h + 1) * 64, off:],
                                lhsT=ones_sb[:, :],
                                rhs=pt_cur[:, h * SCHUNK + off:(h + 1) * SCHUNK],
                                start=(kt == 0), stop=(kt == nkt - 1),
                                tile_position=(0, h * 64),
                                skip_group_check=True)
                        pt_cur = pt_next

                    # normalization, ordered for early PSUM-bank release:
                    # copy raw ctx out first (frees ctx bank for the next
                    # pair), then recip (frees den bank), then normalize in
                    # SBUF off the critical path (consumed by Wo only during
                    # the next q-chunk). The very last pair has no successor
                    # waiting on the banks, so normalize directly from PSUM
                    # (one DVE op shorter -> earlier tail flush).
                    t = ctx_pool.tile([128, SCHUNK], bf16, tag="ctx",
                                      name=f"ctx_{pair}_{qc}")
                    rec = ev_pool.tile([128, SCHUNK], f32, tag="rec", bufs=2)
                    if pair == NPAIR - 1 and qc == nsc - 1:
                        nc.vector.reciprocal_approx_fast(rec[:, :],
                                                         den_p[:, :])
                        nc.vector.tensor_mul(t[:, :], ctx_p[:, :], rec[:, :])
                    else:
                        craw = ev_pool.tile([128, SCHUNK], bf16, tag="craw",
                                            bufs=2)
                        nc.vector.tensor_copy(craw[:, :], ctx_p[:, :])
                        nc.vector.reciprocal_approx_fast(rec[:, :],
                                                         den_p[:, :])
                        nc.vector.tensor_mul(t[:, :], craw[:, :], rec[:, :])
                    ctx_tiles[(pair, qc)] = t

                push_wo(qc)
            flush_all()

    nc.finalize()
    return nc


def _make_masks():
    ki = np.arange(128)[:, None]
    qi = np.arange(SCHUNK)[None, :]
    m = np.stack([(qi >= ki + 128 * r) for r in range(4)]).astype(_BF16)
    return m


def _tile_x(x):
    """[S, D] -> [128, NSC, NDT, SCHUNK] with (p, sc, d, s) = x[sc*512+s,
    d*128+p]; per-partition lines are 8KB contiguous."""
    t = x.T.reshape(NDT, 128, NSC, SCHUNK).transpose(1, 2, 0, 3)
    return np.ascontiguousarray(t).astype(_BF16)


def _host_shards(x_query, x_key, x_value, Wq, Wk, Wv, Wo, s_len):
    """Per-core input dicts. Core c: batch c//2, head-group c%2."""
    masks = _make_masks()
    in_maps = []
    for c in range(8):
        b, g = c // 2, c % 2
        lo, hi = g * CH, (g + 1) * CH
        wqT = Wq[lo:hi, :].T.astype(_BF16)   # [D, CH]
        wkT = Wk[lo:hi, :].T.astype(_BF16)
        wvT = Wv[lo:hi, :].T.astype(_BF16)
        woT = Wo[:, lo:hi].T.astype(_BF16)   # [CH, D]
        in_maps.append({
            "xqT": _tile_x(np.asarray(x_query[b, :s_len], dtype=np.float32)),
            "xkT": _tile_x(np.asarray(x_key[b, :s_len], dtype=np.float32)),
            "xvT": _tile_x(np.asarray(x_value[b, :s_len], dtype=np.float32)),
            # wq: [128, NPAIR, NDT, 128]: (p, m, d, c) = wqT[d*128+p, m*128+c]
            "wqT": np.ascontiguousarray(
                wqT.reshape(NDT, 128, NPAIR, 128).transpose(1, 2, 0, 3)),
            # wk/wv: [128, NDT, CH]: (p, d, c) = wT[d*128+p, c]
            "wkT": np.ascontiguousarray(
                wkT.reshape(NDT, 128, CH).transpose(1, 0, 2)),
            "wvT": np.ascontiguousarray(
                wvT.reshape(NDT, 128, CH).transpose(1, 0, 2)),
            # wo: [128, NPAIR, D]: (p, c, o) = woT[c*128+p, o]
            "woT": np.ascontiguousarray(
                woT.reshape(NPAIR, 128, D).transpose(1, 0, 2)),
            "masks": masks,
        })
    return in_maps


_NC_CACHE = {}


def _get_nc(s_len):
    if s_len not in _NC_CACHE:
        _NC_CACHE[s_len] = _build_nc(s_len)
    return _NC_CACHE[s_len]


def kernel(x_query, x_key, x_value, attention_mask, Wq, Wk, Wv, Wo,
           _trace=False):
    from concourse.bass_utils import run_bass_kernel_spmd

    nc = _get_nc(S)
    in_maps = _host_shards(x_query, x_key, x_value, Wq, Wk, Wv, Wo, S)
    res = run_bass_kernel_spmd(nc, in_maps, core_ids=list(range(8)),
                               trace=_trace)
    y = np.empty((B, S, D), dtype=np.float32)
    for b in range(B):
        y[b] = res.results[2 * b]["y"].astype(np.float32) + \
            res.results[2 * b + 1]["y"].astype(np.float32)
    if _trace:
        return y, res
    return y

